# revision 1
# baseline (speedup 1.0000x reference)
"""BFP8 block quantize-dequantize for Trainium2 (Bass/Tile), 8-core data parallel.

Problem: x (8, 4096, 4096) f32. Each contiguous block of 16 elements (along the
flattened last dims) shares an exponent e = floor(log2(max|x|)); values are
quantized to signed 8-bit mantissas at scale 2^(e-7) and dequantized back.

Sharding: pure data parallel on the leading axis — core c processes x[c]
([4096, 4096] = 64 MiB in, 64 MiB out). No cross-core communication.

Per-core kernel (memory-bound; HBM roofline ~360 GB/s/core -> ~373 us):
  - 16 MiB-contiguous tiles [128 x 4096] f32, triple-plus buffered (bufs=4).
  - Loads issued from SP (sync) HWDGE, stores from ACT (scalar) HWDGE so the
    two directions ride separate queue sets and overlap.
  - VectorE: abs-max reduce over [128, 256, 16] -> block max; exponent bit-math
    (no log2/exp2 needed: for normal floats floor(log2(m)) is the exponent
    field, so scale = 2^(e-7) and rcp = 2^(7-e) are exact bit manipulations);
    quantize q = sat_int8(round(x * rcp)) — the f32->int8 output conversion
    gives round-to-nearest-even + clamp to [-128, 127] for free, which is
    exactly clip(round(.), qmin, qmax).
  - GpSimd: dequantize out = q * scale (int8 x f32-broadcast -> f32), keeping
    VectorE under the DMA roofline.
Zero/denormal blocks: expb clamps to 0 -> scale 0 -> out exactly 0.
"""
import numpy as np

try:
    import concourse.bacc as bacc
except ImportError:  # pragma: no cover - fallback for bare environments
    import sys
    for _p in ("/opt/trn_rl_repo", "/root/.axon_site/_ro/trn_rl_repo"):
        if _p not in sys.path:
            sys.path.insert(0, _p)
    import concourse.bacc as bacc
import concourse.mybir as mybir
import concourse.tile as tile
from concourse.bass_utils import run_bass_kernel_spmd

N_CORES = 8
P = 128                      # SBUF partitions
ROWS, COLS = 4096, 4096      # per-core shard
BLK = 16                     # elements sharing one exponent
MBITS_M1 = 7                 # mantissa_bits - 1
EXP_MASK = 0x7F800000

TILE_F = 4096                # f32 elements per partition per tile
BUFS = 4


def build(reps=1):
    nc = bacc.Bacc()
    x = nc.dram_tensor("x", [ROWS, COLS], mybir.dt.float32, kind="ExternalInput")
    out = nc.dram_tensor("out", [ROWS, COLS], mybir.dt.float32, kind="ExternalOutput")

    total = ROWS * COLS
    n_tiles = total // (P * TILE_F)
    xr = x[:].rearrange("r c -> (r c)").rearrange("(t p f) -> t p f", p=P, f=TILE_F)
    outr = out[:].rearrange("r c -> (r c)").rearrange("(t p f) -> t p f", p=P, f=TILE_F)
    nb = TILE_F // BLK

    with tile.TileContext(nc) as tc:
        with tc.tile_pool(name="sbuf", bufs=BUFS) as pool:
            for t in [t for _ in range(reps) for t in range(n_tiles)]:
                xt = pool.tile([P, TILE_F], mybir.dt.float32, tag="x")
                nc.sync.dma_start(xt[:], xr[t])
                x3 = xt[:].rearrange("p (b k) -> p b k", k=BLK)

                # block max|x|
                bmax = pool.tile([P, nb], mybir.dt.float32, tag="bmax")
                nc.vector.tensor_reduce(
                    bmax[:], x3, axis=mybir.AxisListType.X,
                    op=mybir.AluOpType.max, apply_absolute_value=True,
                )
                # expb = exponent field of bmax == bits of 2^e
                expb = pool.tile([P, nb], mybir.dt.int32, tag="expb")
                nc.vector.tensor_scalar(
                    expb[:], bmax[:].bitcast(mybir.dt.int32),
                    scalar1=EXP_MASK, scalar2=None,
                    op0=mybir.AluOpType.bitwise_and,
                )
                # scale_bits = max(expb, 7<<23) - (7<<23)   [= 2^(e-7); 0 for zero/denormal blocks]
                scaleb = pool.tile([P, nb], mybir.dt.int32, tag="scaleb")
                nc.vector.tensor_scalar(
                    scaleb[:], expb[:],
                    scalar1=(MBITS_M1 << 23), scalar2=-(MBITS_M1 << 23),
                    op0=mybir.AluOpType.max, op1=mybir.AluOpType.add,
                )
                # rcp_bits = (254<<23) - scale_bits         [= 2^(7-e)]
                rcpb = pool.tile([P, nb], mybir.dt.int32, tag="rcpb")
                nc.vector.tensor_scalar(
                    rcpb[:], scaleb[:], scalar1=-1, scalar2=(254 << 23),
                    op0=mybir.AluOpType.mult, op1=mybir.AluOpType.add,
                )
                scale_b = scaleb[:].bitcast(mybir.dt.float32).unsqueeze(2).broadcast_to((P, nb, BLK))
                rcp_b = rcpb[:].bitcast(mybir.dt.float32).unsqueeze(2).broadcast_to((P, nb, BLK))

                # q = sat_int8(round(x * rcp)) == clip(round(x / scale), -128, 127)
                q = pool.tile([P, TILE_F], mybir.dt.int8, tag="q")
                nc.vector.tensor_tensor(
                    q[:].rearrange("p (b k) -> p b k", k=BLK),
                    x3, rcp_b, op=mybir.AluOpType.mult,
                )
                # out = q * scale
                deq = pool.tile([P, TILE_F], mybir.dt.float32, tag="deq")
                nc.gpsimd.tensor_tensor(
                    deq[:].rearrange("p (b k) -> p b k", k=BLK),
                    q[:].rearrange("p (b k) -> p b k", k=BLK),
                    scale_b, op=mybir.AluOpType.mult,
                )
                nc.scalar.dma_start(outr[t], deq[:])
    nc.finalize()
    return nc


_NC_CACHE = {}


def _get_nc(reps=1):
    if reps not in _NC_CACHE:
        _NC_CACHE[reps] = build(reps)
    return _NC_CACHE[reps]


def kernel(x: np.ndarray) -> np.ndarray:
    assert x.shape == (N_CORES, ROWS, COLS) and x.dtype == np.float32, (x.shape, x.dtype)
    nc = _get_nc()
    in_maps = [{"x": np.ascontiguousarray(x[c])} for c in range(N_CORES)]
    res = run_bass_kernel_spmd(nc, in_maps, core_ids=list(range(N_CORES)))
    return np.stack([r["out"] for r in res.results], axis=0)


# revision 2
# speedup vs baseline: 1.1026x; 1.1026x over previous
"""BFP8 block quantize-dequantize for Trainium2 (Bass/Tile), 8-core data parallel.

Problem: x (8, 4096, 4096) f32. Each contiguous block of 16 elements (along the
flattened last dims) shares an exponent e = floor(log2(max|x|)); values are
quantized to signed 8-bit mantissas at scale 2^(e-7) and dequantized back.

Sharding: pure data parallel on the leading axis — core c processes x[c]
([4096, 4096] = 64 MiB in, 64 MiB out). No cross-core communication.

Per-core kernel (memory-bound; HBM roofline ~360 GB/s/core -> ~373 us):
  - 16 MiB-contiguous tiles [128 x 4096] f32, triple-plus buffered (bufs=4).
  - Loads issued from SP (sync) HWDGE, stores from ACT (scalar) HWDGE so the
    two directions ride separate queue sets and overlap.
  - VectorE: abs-max reduce over [128, 256, 16] -> block max; exponent bit-math
    (no log2/exp2 needed: for normal floats floor(log2(m)) is the exponent
    field, so scale = 2^(e-7) and rcp = 2^(7-e) are exact bit manipulations);
    quantize q = sat_int8(round(x * rcp)) — the f32->int8 output conversion
    gives round-to-nearest-even + clamp to [-128, 127] for free, which is
    exactly clip(round(.), qmin, qmax).
  - GpSimd: dequantize out = q * scale (int8 x f32-broadcast -> f32), keeping
    VectorE under the DMA roofline.
Zero/denormal blocks: expb clamps to 0 -> scale 0 -> out exactly 0.
"""
import numpy as np

try:
    import concourse.bacc as bacc
except ImportError:  # pragma: no cover - fallback for bare environments
    import sys
    for _p in ("/opt/trn_rl_repo", "/root/.axon_site/_ro/trn_rl_repo"):
        if _p not in sys.path:
            sys.path.insert(0, _p)
    import concourse.bacc as bacc
import concourse.mybir as mybir
import concourse.tile as tile
from concourse.bass_utils import run_bass_kernel_spmd

N_CORES = 8
P = 128                      # SBUF partitions
ROWS, COLS = 4096, 4096      # per-core shard
BLK = 16                     # elements sharing one exponent
MBITS_M1 = 7                 # mantissa_bits - 1
EXP_MASK = 0x7F800000

TILE_F = 4096                # f32 elements per partition per tile
BUFS = 4


def build(reps=1):
    nc = bacc.Bacc()
    x = nc.dram_tensor("x", [ROWS, COLS], mybir.dt.float32, kind="ExternalInput")
    out = nc.dram_tensor("out", [ROWS, COLS], mybir.dt.float32, kind="ExternalOutput")

    total = ROWS * COLS
    n_tiles = total // (P * TILE_F)
    xr = x[:].rearrange("r c -> (r c)").rearrange("(t p f) -> t p f", p=P, f=TILE_F)
    outr = out[:].rearrange("r c -> (r c)").rearrange("(t p f) -> t p f", p=P, f=TILE_F)
    nb = TILE_F // BLK

    with tile.TileContext(nc) as tc:
        with tc.tile_pool(name="sbuf", bufs=BUFS) as pool:
            for t in [t for _ in range(reps) for t in range(n_tiles)]:
                xt = pool.tile([P, TILE_F], mybir.dt.float32, tag="x")
                nc.sync.dma_start(xt[:], xr[t])
                x3 = xt[:].rearrange("p (b k) -> p b k", k=BLK)

                # block max|x|
                bmax = pool.tile([P, nb], mybir.dt.float32, tag="bmax")
                nc.vector.tensor_reduce(
                    bmax[:], x3, axis=mybir.AxisListType.X,
                    op=mybir.AluOpType.max, apply_absolute_value=True,
                )
                # expb = exponent field of bmax == bits of 2^e
                expb = pool.tile([P, nb], mybir.dt.int32, tag="expb")
                nc.vector.tensor_scalar(
                    expb[:], bmax[:].bitcast(mybir.dt.int32),
                    scalar1=EXP_MASK, scalar2=None,
                    op0=mybir.AluOpType.bitwise_and,
                )
                # scale_bits = max(expb, 7<<23) - (7<<23)   [= 2^(e-7); 0 for zero/denormal blocks]
                scaleb = pool.tile([P, nb], mybir.dt.int32, tag="scaleb")
                nc.vector.tensor_scalar(
                    scaleb[:], expb[:],
                    scalar1=(MBITS_M1 << 23), scalar2=-(MBITS_M1 << 23),
                    op0=mybir.AluOpType.max, op1=mybir.AluOpType.add,
                )
                # rcp_bits = (254<<23) - scale_bits         [= 2^(7-e)]
                rcpb = pool.tile([P, nb], mybir.dt.int32, tag="rcpb")
                nc.vector.tensor_scalar(
                    rcpb[:], scaleb[:], scalar1=-1, scalar2=(254 << 23),
                    op0=mybir.AluOpType.mult, op1=mybir.AluOpType.add,
                )
                scale_b = scaleb[:].bitcast(mybir.dt.float32).unsqueeze(2).broadcast_to((P, nb, BLK))
                rcp_b = rcpb[:].bitcast(mybir.dt.float32).unsqueeze(2).broadcast_to((P, nb, BLK))

                # q = sat_int8(round(x * rcp)) == clip(round(x / scale), -128, 127)
                q = pool.tile([P, TILE_F], mybir.dt.int8, tag="q")
                nc.vector.tensor_tensor(
                    q[:].rearrange("p (b k) -> p b k", k=BLK),
                    x3, rcp_b, op=mybir.AluOpType.mult,
                )
                # out = q * scale
                deq = pool.tile([P, TILE_F], mybir.dt.float32, tag="deq")
                nc.gpsimd.tensor_tensor(
                    deq[:].rearrange("p (b k) -> p b k", k=BLK),
                    q[:].rearrange("p (b k) -> p b k", k=BLK),
                    scale_b, op=mybir.AluOpType.mult,
                )
                nc.scalar.dma_start(outr[t], deq[:])
    nc.finalize()
    return nc


_NC_CACHE = {}


def _get_nc(reps=1):
    if reps not in _NC_CACHE:
        _NC_CACHE[reps] = build(reps)
    return _NC_CACHE[reps]


def kernel(x: np.ndarray) -> np.ndarray:
    x = np.asarray(x)
    assert x.shape == (N_CORES, ROWS, COLS) and x.dtype == np.float32, (x.shape, x.dtype)
    nc = _get_nc()
    in_maps = [{"x": np.ascontiguousarray(x[c])} for c in range(N_CORES)]
    res = run_bass_kernel_spmd(nc, in_maps, core_ids=list(range(N_CORES)))
    return np.stack([r["out"] for r in res.results], axis=0)
